# revision 1
# baseline (speedup 1.0000x reference)
"""Trainium2 Bass kernel for a 2-layer GAT encoder (nn_Encoder_63273458205283).

Strategy (8 NeuronCores, full inputs in / full outputs out):
  - Host: append self-loops, degree-balance nodes into 128-node "windows"
    (49 windows per core), build a global node permutation so each core owns a
    contiguous slot block.  Edge lists per window are padded to a uniform
    K tiles of 128 edges.
  - Device, per core:
      phase0: h_ext = x @ [W1|v_src|v_dst] for ALL nodes -> private DRAM table
              (row = [h0 | 1 | h1 | 1 | a_src | a_dst]), replicated per core.
      layer1: per window: multi-row indirect-DMA gather of source rows,
              per-edge softmax weights ex = exp(leakyrelu(as_src + ad_dst)),
              fold ex into one-hot scatter matrices S (scalar_tensor_tensor),
              PSUM matmul accumulates [weighted-msg | sum(ex)] per dst window;
              normalize by 1/sum at window end (softmax denominator factored
              out of the segment sum), ELU, then h1 @ [W2|v2] -> layer2 table
              rows for own slots.
      AllGather layer2 table shards across the 8 cores.
      layer2: same edge pipeline on the layer2 table; ELU -> output rows.
  - Host: un-permute rows -> h2.  encoded output is x itself.
"""

import math
import os
from dataclasses import dataclass, field

import numpy as np

# ---------------- problem constants (hardcoded; kernel.py is self-contained)
N = 50000
E = 800000
IN = 128
H = 2
C1 = 128          # per-head dim of conv1
C2 = 64           # per-head dim of conv2
NEG_SLOPE = 0.2
NCORES = 8
NEG_BIG = -10000.0  # "as" value of the dummy row -> exp() == 0 for pad edges


@dataclass
class Cfg:
    n_cores: int = NCORES
    n_nodes: int = N
    in_dim: int = IN
    c1: int = C1
    c2: int = C2
    wpc: int = 49              # windows per core
    u_edge: int = 7            # edge-loop unroll (windows per For_i body)
    u0: int = 8                # phase0 unroll (node tiles per body)
    dt_bf16: bool = False      # table/message dtype
    k_tiles: int = 18          # edge tiles (x128) per window; set by prep

    @property
    def spc(self):             # slots per core
        return self.wpc * 128

    @property
    def n_slots(self):
        return self.n_cores * self.spc

    @property
    def ch1(self):
        return self.c1 + 1     # head block incl. the ones column

    @property
    def ch2(self):
        return self.c2 + 1

    @property
    def r1(self):              # layer1 table row length (elements)
        base = 2 * self.ch1 + 4      # h0|1|h1|1|as(2)|ad(2)
        return 272 if self.dt_bf16 else 264 if base <= 264 else base

    @property
    def r2(self):
        base = 2 * self.ch2 + 4
        return 144 if self.dt_bf16 else 136 if base <= 136 else base


# ---------------------------------------------------------------- host prep
def _pack_windows(deg: np.ndarray, n_windows: int) -> list[list[int]]:
    """LPT bin-packing of nodes into n_windows windows of <=128 nodes each,
    balancing total degree per window."""
    import heapq

    order = np.argsort(-deg, kind="stable")
    heap = [(0, w) for w in range(n_windows)]
    heapq.heapify(heap)
    members: list[list[int]] = [[] for _ in range(n_windows)]
    for n in order:
        d = int(deg[n])
        load, w = heapq.heappop(heap)
        members[w].append(int(n))
        if len(members[w]) < 128:
            heapq.heappush(heap, (load + d, w))
    return members


def prep(cfg: Cfg, x, edge_index, W1, att_src1, att_dst1, b1, W2, att_src2,
         att_dst2, b2):
    """All structural + weight preprocessing.  Returns (in_maps, pi)."""
    nn = cfg.n_nodes
    src = np.asarray(edge_index[0], dtype=np.int64)
    dst = np.asarray(edge_index[1], dtype=np.int64)
    loop = np.arange(nn, dtype=np.int64)
    src = np.concatenate([src, loop])
    dst = np.concatenate([dst, loop])

    deg = np.bincount(dst, minlength=nn)
    n_windows = cfg.n_cores * cfg.wpc
    members = _pack_windows(deg, n_windows)

    pi = np.empty(nn, dtype=np.int64)
    pad_slots = []
    for w, mem in enumerate(members):
        for j, n in enumerate(mem):
            pi[n] = w * 128 + j
        for j in range(len(mem), 128):
            pad_slots.append(w * 128 + j)
    pad_slots = np.asarray(pad_slots, dtype=np.int64)

    # edges in slot space (+ self loops for pad slots so 1/sum is finite)
    esrc = np.concatenate([pi[src], pad_slots])
    edst = np.concatenate([pi[dst], pad_slots])
    ew = edst >> 7                      # window id
    eloc = (edst & 127).astype(np.float32)

    order = np.argsort(ew, kind="stable")
    esrc, edst, ew, eloc = esrc[order], edst[order], ew[order], eloc[order]
    counts = np.bincount(ew, minlength=n_windows)
    K = int(math.ceil(counts.max() / 128))
    cfg.k_tiles = K

    starts = np.zeros(n_windows + 1, dtype=np.int64)
    np.cumsum(counts, out=starts[1:])
    j = np.arange(len(esrc)) - starts[ew]          # index within window
    flat = ew * (128 * K) + (j % 128) * K + (j // 128)

    dummy = cfg.n_slots                            # dummy table row index
    srcidx = np.full(n_windows * 128 * K, dummy, dtype=np.int32)
    dstloc = np.zeros(n_windows * 128 * K, dtype=np.float32)
    dstg = np.zeros(n_windows * 128 * K, dtype=np.int32)
    srcidx[flat] = esrc
    dstloc[flat] = eloc
    dstg[flat] = edst
    # pad edges: dstg must be a valid row; point at the window's first slot
    padmask = srcidx == dummy
    wid = (np.arange(n_windows * 128 * K) // (128 * K)).astype(np.int32)
    dstg[padmask] = wid[padmask] * 128

    srcidx = srcidx.reshape(cfg.n_cores, cfg.wpc * 128, K)
    dstloc = dstloc.reshape(cfg.n_cores, cfg.wpc * 128, K)
    dstg = dstg.reshape(cfg.n_cores, cfg.wpc * 128, K)

    # ---- permuted/transposed features
    np_dt = np.float32 if not cfg.dt_bf16 else None
    import ml_dtypes
    np_dt = ml_dtypes.bfloat16 if cfg.dt_bf16 else np.float32
    x = np.asarray(x, dtype=np.float32)
    x_perm = np.zeros((cfg.n_slots, cfg.in_dim), dtype=np.float32)
    x_perm[pi] = x[:nn]
    xT = np.ascontiguousarray(x_perm.T).astype(np_dt)

    # ---- extended weights
    W1 = np.asarray(W1, np.float32)
    W2 = np.asarray(W2, np.float32)
    a_s1 = np.asarray(att_src1, np.float32)
    a_d1 = np.asarray(att_dst1, np.float32)
    a_s2 = np.asarray(att_src2, np.float32)
    a_d2 = np.asarray(att_dst2, np.float32)
    c1, c2, r1, r2 = cfg.c1, cfg.c2, cfg.r1, cfg.r2

    W1h = W1.reshape(cfg.in_dim, H, c1)
    vsrc1 = np.einsum("khc,hc->kh", W1h, a_s1)
    vdst1 = np.einsum("khc,hc->kh", W1h, a_d1)
    wext1 = np.zeros((cfg.in_dim, r1), dtype=np.float32)
    wext1[:, 0:c1] = W1h[:, 0]
    wext1[:, cfg.ch1:cfg.ch1 + c1] = W1h[:, 1]
    wext1[:, 2 * cfg.ch1:2 * cfg.ch1 + 2] = vsrc1
    wext1[:, 2 * cfg.ch1 + 2:2 * cfg.ch1 + 4] = vdst1
    wext1 = wext1.astype(np_dt)

    W2h = W2.reshape(2 * c1, H, c2)
    vsrc2 = np.einsum("khc,hc->kh", W2h, a_s2)
    vdst2 = np.einsum("khc,hc->kh", W2h, a_d2)
    w2full = np.zeros((2 * c1, r2), dtype=np.float32)
    w2full[:, 0:c2] = W2h[:, 0]
    w2full[:, cfg.ch2:cfg.ch2 + c2] = W2h[:, 1]
    w2full[:, 2 * cfg.ch2:2 * cfg.ch2 + 2] = vsrc2
    w2full[:, 2 * cfg.ch2 + 2:2 * cfg.ch2 + 4] = vdst2
    w2ext = np.ascontiguousarray(
        w2full.reshape(2, c1, r2)).astype(np_dt)

    b1r = np.tile(np.asarray(b1, np.float32)[None, :], (128, 1))
    b2r = np.tile(np.asarray(b2, np.float32)[None, :], (128, 1))
    iota = np.tile(np.arange(128, dtype=np.float32)[None, :], (128, 1)).astype(np_dt)
    ident = np.eye(128, dtype=np.float32)

    dummy1 = np.zeros((1, r1), dtype=np.float32)
    dummy1[0, 2 * cfg.ch1:2 * cfg.ch1 + 2] = NEG_BIG
    dummy1 = dummy1.astype(np_dt)
    dummy2 = np.zeros((1, r2), dtype=np.float32)
    dummy2[0, 2 * cfg.ch2:2 * cfg.ch2 + 2] = NEG_BIG
    dummy2 = dummy2.astype(np_dt)

    in_maps = []
    for c in range(cfg.n_cores):
        in_maps.append({
            "xT": xT,
            "wext1": wext1,
            "w2ext": w2ext,
            "b1r": b1r,
            "b2r": b2r,
            "iota": iota,
            "ident": ident,
            "dummy1": dummy1,
            "dummy2": dummy2,
            "srcidx": np.ascontiguousarray(srcidx[c]),
            "dstloc": np.ascontiguousarray(dstloc[c].astype(np_dt)),
            "dstg": np.ascontiguousarray(dstg[c]),
        })
    return in_maps, pi


# ------------------------------------------------------------- bass builder
def build(cfg: Cfg):
    import concourse.bass as bass
    import concourse.bacc as bacc
    import concourse.mybir as mybir
    import concourse.tile as tile
    from concourse.bass import ds

    f32 = mybir.dt.float32
    DT = mybir.dt.bfloat16 if cfg.dt_bf16 else mybir.dt.float32
    i32 = mybir.dt.int32
    Alu = mybir.AluOpType
    Act = mybir.ActivationFunctionType
    ET = mybir.EngineType

    K, U, WPC = cfg.k_tiles, cfg.u_edge, cfg.wpc
    r1, r2, ch1, ch2, c1, c2 = cfg.r1, cfg.r2, cfg.ch1, cfg.ch2, cfg.c1, cfg.c2
    n_slots, spc = cfg.n_slots, cfg.spc

    nc = bacc.Bacc(num_devices=cfg.n_cores)

    # ---- I/O
    xT_d = nc.dram_tensor("xT", [cfg.in_dim, n_slots], DT, kind="ExternalInput")
    wext1_d = nc.dram_tensor("wext1", [cfg.in_dim, r1], DT, kind="ExternalInput")
    w2ext_d = nc.dram_tensor("w2ext", [2, c1, r2], DT, kind="ExternalInput")
    b1r_d = nc.dram_tensor("b1r", [128, 2 * c1], f32, kind="ExternalInput")
    b2r_d = nc.dram_tensor("b2r", [128, 2 * c2], f32, kind="ExternalInput")
    iota_d = nc.dram_tensor("iota", [128, 128], DT, kind="ExternalInput")
    ident_d = nc.dram_tensor("ident", [128, 128], f32, kind="ExternalInput")
    dummy1_d = nc.dram_tensor("dummy1", [1, r1], DT, kind="ExternalInput")
    dummy2_d = nc.dram_tensor("dummy2", [1, r2], DT, kind="ExternalInput")
    srcidx_d = nc.dram_tensor("srcidx", [spc, K], i32, kind="ExternalInput")
    dstloc_d = nc.dram_tensor("dstloc", [spc, K], DT, kind="ExternalInput")
    dstg_d = nc.dram_tensor("dstg", [spc, K], i32, kind="ExternalInput")
    out2_d = nc.dram_tensor("out2", [spc, 2 * c2], f32, kind="ExternalOutput")
    debug_taps = bool(int(os.environ.get("GAT_DEBUG_TAPS", "0")))
    if debug_taps:
        dbg1_d = nc.dram_tensor("dbg_table1", [n_slots + 1, r1], DT,
                                kind="ExternalOutput")
        dbg2_d = nc.dram_tensor("dbg_h2table", [n_slots + 1, r2], DT,
                                kind="ExternalOutput")

    table1 = nc.dram_tensor("table1", [n_slots + 1, r1], DT, kind="Internal")
    h2shard = nc.dram_tensor("h2shard", [spc, r2], DT, kind="Internal")
    h2table = nc.dram_tensor("h2table", [n_slots + 1, r2], DT, kind="Internal")

    hint = (ET.DVE, ET.PE, ET.Activation)

    with tile.TileContext(nc) as tc:
        with (
            tc.tile_pool(name="const", bufs=1) as cpool,
            tc.tile_pool(name="work", bufs=3) as wpool,
            tc.tile_pool(name="small", bufs=6) as spool,
            tc.tile_pool(name="psum", bufs=2, space="PSUM") as ppool,
        ):
            # ---- load constants
            wext1_sb = cpool.tile([cfg.in_dim, r1], DT, tag="wext1")
            nc.sync.dma_start(wext1_sb[:], wext1_d[:, :])
            w2ext_sb = cpool.tile([c1, 2, r2], DT, tag="w2ext")
            nc.sync.dma_start(
                w2ext_sb[:], w2ext_d[:, :, :].rearrange("b p c -> p b c"))
            b1r_sb = cpool.tile([128, 2 * c1], f32, tag="b1r")
            nc.sync.dma_start(b1r_sb[:], b1r_d[:, :])
            b2r_sb = cpool.tile([128, 2 * c2], f32, tag="b2r")
            nc.sync.dma_start(b2r_sb[:], b2r_d[:, :])
            iota_sb = cpool.tile([128, 128], DT, tag="iota")
            nc.sync.dma_start(iota_sb[:], iota_d[:, :])
            ident_sb = cpool.tile([128, 128], f32, tag="ident")
            nc.sync.dma_start(ident_sb[:], ident_d[:, :])

            # ---- dummy rows
            dr1 = cpool.tile([1, r1], DT, tag="dr1")
            nc.sync.dma_start(dr1[:], dummy1_d[:, :])
            nc.sync.dma_start(table1[n_slots:n_slots + 1, :], dr1[:])
            dr2 = cpool.tile([1, r2], DT, tag="dr2")
            nc.sync.dma_start(dr2[:], dummy2_d[:, :])
            nc.sync.dma_start(h2table[n_slots:n_slots + 1, :], dr2[:])

            # ---- phase 0: full layer1 table, replicated on every core
            t0 = n_slots // 128
            assert t0 % cfg.u0 == 0
            if "0" not in os.environ.get("GAT_SKIP", ""):
              u0 = cfg.u0
              with tc.For_i(0, n_slots, u0 * 128, hint_engines=hint) as i0:
                  xsl = wpool.tile([cfg.in_dim, u0 * 128], DT, tag="xsl")
                  nc.sync.dma_start(xsl[:], xT_d[:, ds(i0, u0 * 128)])
                  rsl = wpool.tile([128, u0, r1], DT, tag="rsl")
                  for u in range(u0):
                      ps0 = ppool.tile([128, r1], f32, tag="ps0", bufs=2)
                      nc.tensor.matmul(ps0[:], lhsT=xsl[:, u * 128:(u + 1) * 128],
                                       rhs=wext1_sb[:], start=True, stop=True)
                      nc.vector.tensor_copy(rsl[:, u, :], ps0[:])
                      nc.vector.memset(rsl[:, u, c1:c1 + 1], 1.0)
                      nc.vector.memset(rsl[:, u, ch1 + c1:ch1 + c1 + 1], 1.0)
                  nc.sync.dma_start(
                      table1[ds(i0, u0 * 128), :].rearrange(
                          "(u p) c -> p u c", p=128), rsl[:])

            # ---- shared edge phase
            def edge_phase(table, R, C, CH, bias_sb, finish):
                as_off = 2 * CH
                ad_off = 2 * CH + 2
                with tc.For_i(0, spc, U * 128, hint_engines=hint) as iw:
                    srcsl = wpool.tile([128, U, K], i32, tag="srcsl")
                    nc.sync.dma_start(
                        srcsl[:],
                        srcidx_d[ds(iw, U * 128), :].rearrange(
                            "(u p) k -> p u k", p=128))
                    locsl = wpool.tile([128, U, K], DT, tag="locsl")
                    nc.sync.dma_start(
                        locsl[:],
                        dstloc_d[ds(iw, U * 128), :].rearrange(
                            "(u p) k -> p u k", p=128))
                    dstgsl = wpool.tile([128, U, K], i32, tag="dstgsl")
                    nc.sync.dma_start(
                        dstgsl[:],
                        dstg_d[ds(iw, U * 128), :].rearrange(
                            "(u p) k -> p u k", p=128))
                    osl = wpool.tile([128, U, finish.out_w], finish.out_dt,
                                     tag="osl")
                    for u in range(U):
                        gath = wpool.tile([128, K, R], DT, tag="gath")
                        for k in range(K):
                            nc.gpsimd.indirect_dma_start(
                                out=gath[:, k, :], out_offset=None,
                                in_=table[:, :],
                                in_offset=bass.IndirectOffsetOnAxis(
                                    ap=srcsl[:, u, k:k + 1], axis=0))
                        adg = wpool.tile([128, K, 2], DT, tag="adg")
                        for k in range(K):
                            nc.gpsimd.indirect_dma_start(
                                out=adg[:, k, :], out_offset=None,
                                in_=table[:, :],
                                in_offset=bass.IndirectOffsetOnAxis(
                                    ap=dstgsl[:, u, k:k + 1], axis=0),
                                element_offset=ad_off)
                        acc0 = ppool.tile([128, CH], f32, tag="acc0", bufs=2)
                        acc1 = ppool.tile([128, CH], f32, tag="acc1", bufs=2)
                        acc = [acc0, acc1]
                        for k in range(K):
                            e_t = spool.tile([128, 2], f32, tag="e")
                            nc.vector.tensor_tensor(
                                out=e_t[:],
                                in0=gath[:, k, as_off:as_off + 2],
                                in1=adg[:, k, :], op=Alu.add)
                            lr_t = spool.tile([128, 2], f32, tag="lr")
                            nc.vector.scalar_tensor_tensor(
                                out=lr_t[:], in0=e_t[:], scalar=NEG_SLOPE,
                                in1=e_t[:], op0=Alu.mult, op1=Alu.max)
                            ex_t = spool.tile([128, 2], DT, tag="ex")
                            nc.scalar.activation(out=ex_t[:], in_=lr_t[:],
                                                 func=Act.Exp)
                            for h in range(2):
                                s_t = spool.tile([128, 128], DT, tag=f"S{h}")
                                nc.vector.scalar_tensor_tensor(
                                    out=s_t[:], in0=iota_sb[:],
                                    scalar=locsl[:, u, k:k + 1],
                                    in1=ex_t[:, h:h + 1].to_broadcast(
                                        [128, 128]),
                                    op0=Alu.is_equal, op1=Alu.mult)
                                nc.tensor.matmul(
                                    acc[h][:], lhsT=s_t[:],
                                    rhs=gath[:, k, h * CH:(h + 1) * CH],
                                    start=(k == 0), stop=(k == K - 1))
                        # window epilogue: normalize + bias + ELU
                        recip = spool.tile([128, 2], f32, tag="recip")
                        for h in range(2):
                            nc.vector.reciprocal(
                                recip[:, h:h + 1],
                                acc[h][:, CH - 1:CH])
                        ob = spool.tile([128, 2 * C], f32, tag="ob")
                        for h in range(2):
                            nc.vector.scalar_tensor_tensor(
                                out=ob[:, h * C:(h + 1) * C],
                                in0=acc[h][:, 0:C],
                                scalar=recip[:, h:h + 1],
                                in1=bias_sb[:, h * C:(h + 1) * C],
                                op0=Alu.mult, op1=Alu.add)
                        ee = spool.tile([128, 2 * C], f32, tag="ee")
                        nc.scalar.activation(out=ee[:], in_=ob[:], func=Act.Exp)
                        nc.vector.tensor_scalar_sub(ee[:], ee[:], 1.0)
                        mk = spool.tile([128, 2 * C], mybir.dt.uint8, tag="mk")
                        nc.vector.tensor_scalar(mk[:], ob[:], 0.0, scalar2=None,
                                                op0=Alu.is_gt)
                        nc.vector.copy_predicated(ee[:], mk[:], ob[:])
                        finish.emit(u, ee, osl)
                    finish.store(iw, osl)

            # ---- layer1 finish: build layer2 table rows for own slots
            class Fin1:
                out_w = r2
                out_dt = DT

                def emit(self, u, ee, osl):
                    h1T = []
                    for b in range(2):
                        pst = ppool.tile([128, 128], f32, tag="pst", bufs=1)
                        nc.tensor.transpose(pst[:], ee[:, b * 128:(b + 1) * 128],
                                            ident_sb[:])
                        ht = wpool.tile([128, 128], DT, tag=f"h1T{b}")
                        nc.vector.tensor_copy(ht[:], pst[:])
                        h1T.append(ht)
                    h2p = ppool.tile([128, r2], f32, tag="h2p", bufs=1)
                    nc.tensor.matmul(h2p[:], lhsT=h1T[0][:],
                                     rhs=w2ext_sb[:, 0, :], start=True,
                                     stop=False)
                    nc.tensor.matmul(h2p[:], lhsT=h1T[1][:],
                                     rhs=w2ext_sb[:, 1, :], start=False,
                                     stop=True)
                    nc.vector.tensor_copy(osl[:, u, :], h2p[:])
                    nc.vector.memset(osl[:, u, c2:c2 + 1], 1.0)
                    nc.vector.memset(osl[:, u, ch2 + c2:ch2 + c2 + 1], 1.0)

                def store(self, iw, osl):
                    nc.sync.dma_start(
                        h2shard[ds(iw, U * 128), :].rearrange(
                            "(u p) c -> p u c", p=128), osl[:])

            # ---- layer2 finish: final output rows (f32)
            class Fin2:
                out_w = 2 * c2
                out_dt = f32

                def emit(self, u, ee, osl):
                    nc.vector.tensor_copy(osl[:, u, :], ee[:])

                def store(self, iw, osl):
                    nc.sync.dma_start(
                        out2_d[ds(iw, U * 128), :].rearrange(
                            "(u p) c -> p u c", p=128), osl[:])

            if debug_taps:
                for cc in range(0, n_slots + 1, 3136):
                    ce = min(cc + 3136, n_slots + 1)
                    nc.sync.dma_start(dbg1_d[cc:ce, :], table1[cc:ce, :])
            if "1" not in os.environ.get("GAT_SKIP", ""):
                edge_phase(table1, r1, c1, ch1, b1r_sb, Fin1())

            if "c" not in os.environ.get("GAT_SKIP", ""):
                nc.gpsimd.collective_compute(
                    kind="AllGather", op=mybir.AluOpType.bypass,
                    replica_groups=[list(range(cfg.n_cores))],
                    ins=[h2shard[:, :]], outs=[h2table[0:n_slots, :]])

            if debug_taps:
                for cc in range(0, n_slots + 1, 3136):
                    ce = min(cc + 3136, n_slots + 1)
                    nc.sync.dma_start(dbg2_d[cc:ce, :], h2table[cc:ce, :])
            if "2" not in os.environ.get("GAT_SKIP", ""):
                edge_phase(h2table, r2, c2, ch2, b2r_sb, Fin2())

    nc.finalize()
    return nc


# ------------------------------------------------------------------ driver
_CACHE: dict = {}


def kernel(x, edge_index, W1, att_src1, att_dst1, b1, W2, att_src2, att_dst2,
           b2):
    from concourse.bass_utils import run_bass_kernel_spmd

    cfg = Cfg(dt_bf16=bool(int(os.environ.get("GAT_BF16", "0"))))
    in_maps, pi = prep(cfg, x, edge_index, W1, att_src1, att_dst1, b1, W2,
                       att_src2, att_dst2, b2)
    key = (cfg.k_tiles, cfg.dt_bf16)
    if key not in _CACHE:
        _CACHE[key] = build(cfg)
    nc = _CACHE[key]
    res = run_bass_kernel_spmd(nc, in_maps, core_ids=list(range(cfg.n_cores)))
    out = np.concatenate([res.results[c]["out2"] for c in range(cfg.n_cores)],
                         axis=0)
    h2 = np.ascontiguousarray(out[pi[:cfg.n_nodes]], dtype=np.float32)
    encoded = np.asarray(x, dtype=np.float32)
    return (h2, encoded)

